# revision 39
# baseline (speedup 1.0000x reference)
"""Trainium2 Bass kernel for DifferentiableSupergraphDynamics.

Computation:
    edge_w = where(learn_mask, tanh(theta), sign*conf) * delay      [E]
    msgs   = x[:, src] * edge_w                                     [B, E]
    agg    = scatter_add(msgs -> dst)                               [B, N]
    rate   = base_rate * exp(rate_log_scale)                        [N]
    drive  = tanh(agg + bias)
    x_next = clip(x + DT * rate * (drive*cap - x), 0, cap)

Sharding: destination nodes are dealt round-robin (by total in-degree
rank) across the 8 cores; every edge lives on its destination's core, so
no cross-core collective is needed.

Per-core edge phase: edges are split into 4 "structures" by source-node
range (32768 rows each, so dma_gather's int16 indices can address the x
table). Each structure is a padded CSR over the core's nodes sorted by
that structure's in-degree: node groups of 128 partitions padded to the
group max degree D. Structures are processed as group-aligned chunks of
<= 8192 slots; each chunk is one SWDGE dma_gather call. Calls round-robin
the 4 SWDGE queues; with the descriptor ring doubled (32KB carveout) a
queue's next generation overlaps the previous call's drain, keeping all
8 Q7 cores generating descriptors continuously (~66us per 8192-desc call
per queue). Per-chunk Vector work (weight multiply + strided
tensor_reduce) and the per-structure merge scatter-adds (into canonical
node order via HBM) ride under the SWDGE generation critical path.
"""

import os

import numpy as np

# insurance against a wedged device left by a previous process
os.environ.setdefault("NEURON_RT_RESET_CORES", "1")

import concourse.bass as bass
import concourse.bacc as bacc
import concourse.mybir as mybir
import concourse.tile as tile
from concourse.bass_utils import run_bass_kernel_spmd

P = 128
NCORES = 8
DT = 0.1
SRC_CHUNK = 32768          # dma_gather int16 index reach
CALL_SLOTS = 8192          # max slots per gather call (<= ring capacity)
CALL_COLS = CALL_SLOTS // P
RING_BYTES = 32768         # SWDGE descriptor carveout (2 calls in flight)
ROWE = 64                  # x-table row stride: 256B (dma_gather req)

F32 = mybir.dt.float32
I16 = mybir.dt.int16
I8 = mybir.dt.int8


def _wrap_idx(flat):
    """SWDGE wrapped index layout for one call: index j at [j%16, j//16],
    replicated to 128 partitions (each queue's Q7 pair reads its own
    16-partition stripe)."""
    n = len(flat)
    assert n % 16 == 0
    cols = flat.reshape(n // 16, 16).T            # [16, n/16]
    return np.concatenate([cols] * 8, axis=0)     # [128, n/16]


# ---------------------------------------------------------------------------
# Host-side data preparation
# ---------------------------------------------------------------------------

def _chunk_groups(D_q, gact):
    """Split active groups [0, gact) into runs with sum(D) <= CALL_COLS."""
    chunks = []
    g = 0
    while g < gact:
        g2 = g
        tot = 0
        while g2 < gact and tot + D_q[g2] <= CALL_COLS:
            tot += D_q[g2]
            g2 += 1
        assert g2 > g
        chunks.append((g, g2, int(tot)))
        g = g2
    return chunks


def _prep(x, theta, bias, ratelog, baserate, cap, sign, conf, delay, src, dst,
          mask, n_cores):
    B, N = x.shape
    E = src.shape[0]

    src = np.asarray(src).astype(np.int64)
    dst = np.asarray(dst).astype(np.int64)
    theta = np.asarray(theta, dtype=np.float32)
    sign = np.asarray(sign, dtype=np.float32)
    conf = np.asarray(conf, dtype=np.float32)
    delay = np.asarray(delay, dtype=np.float32)
    mask8 = np.asarray(mask).astype(np.int8)

    deg = np.bincount(dst, minlength=N)
    order = np.argsort(-deg, kind="stable")
    npc = (N + n_cores - 1) // n_cores
    G = (npc + P - 1) // P
    nper = G * P                                   # nodes per core (padded)

    rank_of = np.empty(N, dtype=np.int64)
    rank_of[order] = np.arange(N)
    core_of = rank_of % n_cores                    # node -> core
    pos_of = rank_of // n_cores                    # node -> position in core

    n_pad = ((N + ROWE - 1) // ROWE) * ROWE
    nq = (n_pad + SRC_CHUNK - 1) // SRC_CHUNK     # structures
    q_of = src // SRC_CHUNK                        # edge -> structure

    # per (core, structure) in-degree
    edge_core = core_of[dst]
    edge_pos = pos_of[dst]
    degq = np.zeros((n_cores, nper, nq), dtype=np.int64)
    np.add.at(degq, (edge_core, edge_pos, q_of), 1)

    # shared-over-cores placement per structure: within each core sort
    # positions by degq desc; group windows of 128; D = max over cores.
    D = np.zeros((nq, G), dtype=np.int64)
    ordq = np.zeros((n_cores, nq, nper), dtype=np.int64)   # row j -> position
    invq = np.zeros((n_cores, nq, nper), dtype=np.int64)   # position -> row j
    for q in range(nq):
        for c in range(n_cores):
            o = np.argsort(-degq[c, :, q], kind="stable")
            ordq[c, q] = o
            invq[c, q, o] = np.arange(nper)
            dm = degq[c, o, q].reshape(G, P).max(axis=1)
            D[q] = np.maximum(D[q], dm)
    D[0] = np.maximum(D[0], 1)       # canonical layout covers all nodes
    S = np.zeros((nq, G + 1), dtype=np.int64)
    S[:, 1:] = np.cumsum(D, axis=1)
    F = S[:, -1]                                   # cols per structure
    Gact = np.array([int((D[q] > 0).sum()) for q in range(nq)])

    # --- edge slot assignment ---
    eord = np.lexsort((src, dst))
    ec = edge_core[eord]
    ep = edge_pos[eord]
    eq = q_of[eord]
    key_change = np.ones(E, dtype=bool)
    key_change[1:] = (dst[eord][1:] != dst[eord][:-1]) | (eq[1:] != eq[:-1])
    run_id = np.cumsum(key_change) - 1
    run_starts = np.flatnonzero(key_change)
    occ = np.arange(E) - run_starts[run_id]

    row = invq[ec, eq, ep]                         # row index in structure
    g = row // P
    pp = row % P
    col = S[eq, g] + occ
    slot_i = pp + P * col                          # slot within (core, struct)

    FT = int(F.sum())
    Scol = np.zeros(nq + 1, dtype=np.int64)
    Scol[1:] = np.cumsum(F)

    # edge params laid out [P, FT] per core (slot (q,p,col) -> [p,Scol[q]+col])
    par_shape = (n_cores, P, FT)
    thetaA = np.zeros(par_shape, np.float32)
    signA = np.zeros(par_shape, np.float32)
    confA = np.zeros(par_shape, np.float32)
    delayA = np.zeros(par_shape, np.float32)
    maskA = np.zeros(par_shape, np.int8)
    pidx = (ec, pp, Scol[eq] + col)
    thetaA[pidx] = theta[eord]
    signA[pidx] = sign[eord]
    confA[pidx] = conf[eord]
    delayA[pidx] = delay[eord]
    maskA[pidx] = mask8[eord]

    # --- chunk plans (shared across cores: D is shared) ---
    chunks = [_chunk_groups(D[q], int(Gact[q])) for q in range(nq)]

    def _split_chunk(q, ch):
        g0, g1, cols = ch
        if g1 - g0 < 2:
            return [ch]
        best, bc = g0 + 1, None
        for gm in range(g0 + 1, g1):
            c = int(D[q][g0:gm].sum())
            if bc is None or abs(c - cols / 2) < abs(bc - cols / 2):
                best, bc = gm, c
        return [(g0, best, bc), (best, g1, cols - bc)]

    # halve the first round's chunks (faster pipeline fill) and the last
    # two of the final structure (finer tail balancing)
    chunks[1] = (_split_chunk(1, chunks[1][0]) + _split_chunk(1, chunks[1][1])
                 + chunks[1][2:])
    chunks[0] = (chunks[0][:-2] + _split_chunk(0, chunks[0][-2])
                 + _split_chunk(0, chunks[0][-1]))

    # --- merge-gather blocks: canonical-order re-gather of each partial
    # aggregate, split into ~25-col blocks (slot counts multiple of 128) ---
    def _merge_blocks(gtot):
        nb = max(1, (gtot + 24) // 25)
        base, rem = divmod(gtot, nb)
        out, g = [], 0
        for i in range(nb):
            w = base + (1 if i < rem else 0)
            out.append((g, g + w))
            g += w
        return out

    sblocks = {q: _merge_blocks(G) for q in range(1, nq)}

    # emission schedule: gathers for structures in qorder, with each
    # structure's merge-scatter chunks emitted after the NEXT structure's
    # gathers (so their reduce deps are long satisfied at dispatch).
    # qorder = [1, 2, 3, 0]; scatter(1) after G2, scatter(3) after G3... etc.
    # gather emission order: structures 1, 2, 3, 0
    gorder = ([("g", 1, i) for i in range(len(chunks[1]))] +
              [("g", 2, i) for i in range(len(chunks[2]))] +
              [("g", 3, i) for i in range(len(chunks[3]))] +
              [("g", 0, i) for i in range(len(chunks[0]))])
    gpos = {(q, ci): j for j, (_, q, ci) in enumerate(gorder)}

    # merge-gather blocks depend on the whole structure's reduces plus its
    # agg HBM writeback; emit them LAG gathers after the structure's last
    # chunk so those (which trail the drain by ~1 round) never stall SEQ
    # dispatch.
    LAG = 8
    pend = []
    for q in range(1, nq):
        last = gpos[(q, len(chunks[q]) - 1)]
        for bi in range(len(sblocks[q])):
            pend.append((last + LAG + bi, ("s", q, bi)))
    pend.sort(key=lambda e: e[0])

    sched = []                                     # (kind, q, idx, queue)
    pi = 0
    load = [0, 0, 0, 0]

    def _emit(ev):
        kind, q, ci = ev
        if kind == "g":
            n = chunks[q][ci][2] * P
        else:
            g0, g1 = sblocks[q][ci]
            n = (g1 - g0) * P
        qu = min(range(4), key=lambda r: (load[r], r))
        load[qu] += n
        sched.append((kind, q, ci, qu))

    for j, gev in enumerate(gorder):
        while pi < len(pend) and pend[pi][0] <= j:
            _emit(pend[pi][1])
            pi += 1
        _emit(gev)
    for _, ev in pend[pi:]:
        _emit(ev)

    # --- per-core gather index blobs, laid out in EMISSION order so the
    # first rounds' indices can be loaded (and gathered from) first ---
    srcrel = (src[eord] - eq * SRC_CHUNK).astype(np.int16)
    amaps = {}
    for q in range(nq):
        tots = int(F[q]) * P
        for c in range(n_cores):
            a = np.zeros(tots, np.int16)
            selq = (ec == c) & (eq == q)
            a[slot_i[selq]] = srcrel[selq]
            amaps[(q, c)] = a
    gcol0 = {}                                     # (q, ci) -> gidx col offset
    gidx_parts = [[] for _ in range(n_cores)]
    colp = 0
    gidx_split = 0
    for j, (_, q, ci) in enumerate(gorder):
        g0, g1, cols = chunks[q][ci]
        c0, c1 = int(S[q, g0]), int(S[q, g1])
        gcol0[(q, ci)] = colp
        for c in range(n_cores):
            gidx_parts[c].append(_wrap_idx(amaps[(q, c)][c0 * P:c1 * P]))
        colp += (c1 - c0) * P // 16
        if j == 3:
            gidx_split = colp                      # first round loads alone
    gidx = [np.concatenate(p, axis=1) for p in gidx_parts]
    gidx_cols = gidx[0].shape[1]

    # --- per-core merge-gather index blobs: for canonical output slot
    # (p, g) the structure-q table row of its node; table row layout is
    # (jq % P) * G + jq // P where jq is the node's structure-q row ---
    scol0 = {}
    sidx_parts = [[] for _ in range(n_cores)]
    colp = 0
    qrow_of = np.zeros((n_cores, nq, nper), np.int16)   # canonical row->q row
    for q in range(1, nq):
        for c in range(n_cores):
            node_pos = ordq[c, 0]                  # canonical row -> position
            jq = invq[c, q, node_pos]              # -> structure-q row
            qrow_of[c, q] = ((jq % P) * G + (jq // P)).astype(np.int16)
    for q in range(1, nq):
        for ci, (g0, g1) in enumerate(sblocks[q]):
            scol0[(q, ci)] = colp
            nact = (g1 - g0) * P
            for c in range(n_cores):
                # output slot s = p + 128*(g-g0) -> canonical row g*P + p,
                # which is arange(g0*P, g1*P) in slot order
                crow = np.arange(g0 * P, g1 * P)
                sidx_parts[c].append(_wrap_idx(qrow_of[c, q, crow]))
            colp += nact // 16
    sidx = [np.concatenate(p, axis=1) for p in sidx_parts]
    sidx_cols = sidx[0].shape[1]

    # node params in canonical placement [P, G]
    def node_arr(vals, fill):
        a = np.full((n_cores, P, G), fill, dtype=np.float32)
        for c in range(n_cores):
            node_pos = ordq[c, 0]
            rank = node_pos * n_cores + c
            ok = rank < N
            nd = order[np.minimum(rank, N - 1)]
            v = np.where(ok, vals[nd], fill).astype(np.float32)
            a[c].reshape(-1)[(np.arange(nper) % P) * G +
                             (np.arange(nper) // P)] = np.where(ok, v, fill)
        return a

    biasA = node_arr(np.asarray(bias), 0.0)
    ratelogA = node_arr(np.asarray(ratelog), 0.0)
    baserateA = node_arr(np.asarray(baserate), 0.0)
    capA = node_arr(np.asarray(cap), 1.0)

    xT4 = np.zeros((n_pad, ROWE), np.float32)
    xT4[:N, :B] = np.asarray(x, dtype=np.float32).T

    xTf = xT4[:, :B]
    xownA = np.zeros((n_cores, P, G, B), np.float32)
    node_ids = np.zeros((n_cores, P, G), np.int64)
    for c in range(n_cores):
        node_pos = ordq[c, 0]
        rank = node_pos * n_cores + c
        ok = rank < N
        nd = np.where(ok, order[np.minimum(rank, N - 1)], -1)
        jj = np.arange(nper)
        pcol = (jj % P, jj // P)
        node_ids[c][pcol] = nd
        xownA[c][pcol[0], pcol[1], :] = np.where(
            ok[:, None], xTf[np.maximum(nd, 0), :], 0.0)

    ins = []
    for c in range(n_cores):
        ins.append({
            "xT4": xT4,
            "gidx": gidx[c],
            "sidx": sidx[c],
            "theta": thetaA[c],
            "sgn": signA[c],
            "conf": confA[c],
            "delay": delayA[c],
            "maskf": maskA[c],
            "bias": biasA[c],
            "ratelog": ratelogA[c],
            "baserate": baserateA[c],
            "cap": capA[c],
            "xown": xownA[c].reshape(P, G * B),
        })
    plan = dict(B=B, N=N, G=G, nq=nq, D=D, S=S, F=F, Scol=Scol, Gact=Gact,
                n_pad=n_pad, gidx_cols=gidx_cols, sidx_cols=sidx_cols,
                chunks=chunks, sblocks=sblocks, sched=sched, gcol0=gcol0,
                scol0=scol0, gidx_split=gidx_split, node_ids=node_ids)
    return ins, plan


def _assemble(results, plan):
    B, N, G = plan["B"], plan["N"], plan["G"]
    out = np.empty((B, N), dtype=np.float32)
    for ci, res in enumerate(results):
        o = res["out"].reshape(P, G, B)
        nid = plan["node_ids"][ci]
        ok = nid >= 0
        out[:, nid[ok]] = o[ok].T
    return out


# ---------------------------------------------------------------------------
# Device kernel
# ---------------------------------------------------------------------------

def _raw_dma_gather(g, out_ap, in_ap, idxs_ap, num_idxs, num_idxs_reg,
                    elem_size, elem_step, queue_num):
    stride_bytes = elem_step * mybir.dt.size(in_ap.dtype)
    return g.add_instruction(
        mybir.InstDMAGatherAnt(
            name=g.bass.get_next_instruction_name(),
            ins=[*g.lower_ap_dma(in_ap, for_custom_bir_dma=True),
                 g.lower_ap(idxs_ap), g.lower_val_access(num_idxs_reg)],
            outs=[g.lower_ap(out_ap)],
            transpose=False, num_idxs=num_idxs, elem_size=elem_size,
            stride_bytes_256=stride_bytes // 256, gen_mode=0,
            single_packet=False, queue_num=queue_num,
            sbuf_tokens_per_rank=0, sbuf_free_dim_per_rank=0,
            sbuf_free_dim_pad_per_rank=0, sbuf_byte_offset=0))


def _equal_d_runs(D, g0, g1):
    runs = []
    a = g0
    while a < g1:
        b = a + 1
        while b < g1 and D[b] == D[a]:
            b += 1
        runs.append((a, b, int(D[a])))
        a = b
    return runs


def build(plan):
    B = plan["B"]
    G = plan["G"]
    nq = plan["nq"]
    D, S, F, Scol = plan["D"], plan["S"], plan["F"], plan["Scol"]
    n_pad = plan["n_pad"]
    chunks, sched = plan["chunks"], plan["sched"]
    sblocks = plan["sblocks"]
    gcol0, scol0 = plan["gcol0"], plan["scol0"]
    FT = int(Scol[-1])

    # distinct SWDGE call sizes -> one shared register each (written once;
    # per-call to_reg MOVEs rewrite a single GPR and Tile serializes every
    # call behind the previous one's completion sem to protect it)
    sizes = set()
    for kind, q, ci, _qu in sched:
        if kind == "g":
            sizes.add(chunks[q][ci][2] * P)
        else:
            g0, g1 = sblocks[q][ci]
            sizes.add((g1 - g0) * P)

    nc = bacc.Bacc("TRN2", target_bir_lowering=False, debug=False,
                   enable_asserts=False, num_swdge_queues=4,
                   dynamic_dma_scratch_size=RING_BYTES)

    xT4 = nc.dram_tensor("xT4", [n_pad, ROWE], F32, kind="ExternalInput")
    giD = nc.dram_tensor("gidx", [128, plan["gidx_cols"]], I16,
                         kind="ExternalInput")
    siD = nc.dram_tensor("sidx", [128, plan["sidx_cols"]], I16,
                         kind="ExternalInput")
    thD = nc.dram_tensor("theta", [P, FT], F32, kind="ExternalInput")
    sgD = nc.dram_tensor("sgn", [P, FT], F32, kind="ExternalInput")
    cfD = nc.dram_tensor("conf", [P, FT], F32, kind="ExternalInput")
    dlD = nc.dram_tensor("delay", [P, FT], F32, kind="ExternalInput")
    mkD = nc.dram_tensor("maskf", [P, FT], I8, kind="ExternalInput")
    biD = nc.dram_tensor("bias", [P, G], F32, kind="ExternalInput")
    rlD = nc.dram_tensor("ratelog", [P, G], F32, kind="ExternalInput")
    brD = nc.dram_tensor("baserate", [P, G], F32, kind="ExternalInput")
    cpD = nc.dram_tensor("cap", [P, G], F32, kind="ExternalInput")
    xoD = nc.dram_tensor("xown", [P, G * B], F32, kind="ExternalInput")
    outD = nc.dram_tensor("out", [P, G * B], F32, kind="ExternalOutput")
    # partial-agg merge staging tables (HBM roundtrip for the canonical
    # re-gather)
    pagg = [nc.dram_tensor(f"pagg{q}", [G * P, ROWE], F32,
                           kind="ExternalOutput") for q in range(1, nq)]

    Tanh = mybir.ActivationFunctionType.Tanh
    Exp = mybir.ActivationFunctionType.Exp

    with tile.TileContext(nc) as tc:
        with tc.tile_pool(name="persist", bufs=1) as pp:
            nreg = {v: nc.gpsimd.to_reg(v) for v in sorted(sizes)}

            # first round's gather indices load alone so gathers start early
            spl = int(plan["gidx_split"])
            gidxA = pp.tile([128, spl], I16, tag="gidxA")
            nc.sync.dma_start(out=gidxA[:], in_=giD[:, :spl])
            gidxB = pp.tile([128, plan["gidx_cols"] - spl], I16, tag="gidxB")
            nc.sync.dma_start(out=gidxB[:], in_=giD[:, spl:])
            sidx_t = pp.tile([128, plan["sidx_cols"]], I16, tag="sidx")
            nc.sync.dma_start(out=sidx_t[:], in_=siD[:, :])

            bi = pp.tile([P, G], F32, tag="bi")
            rl = pp.tile([P, G], F32, tag="rl")
            br = pp.tile([P, G], F32, tag="br")
            cp = pp.tile([P, G], F32, tag="cp")
            xo = pp.tile([P, G * B], F32, tag="xo")
            nc.sync.dma_start(out=bi[:], in_=biD[:, :])
            nc.sync.dma_start(out=rl[:], in_=rlD[:, :])
            nc.sync.dma_start(out=br[:], in_=brD[:, :])
            nc.sync.dma_start(out=cp[:], in_=cpD[:, :])
            nc.sync.dma_start(out=xo[:], in_=xoD[:, :])

            w = pp.tile([P, FT], F32, tag="w")
            agg0 = pp.tile([P, G * B], F32, tag="agg0")
            aggq = {}
            canq = {}
            for q in range(1, nq):
                aggq[q] = pp.tile([P, G * B], F32, tag=f"agg{q}",
                                  name=f"aggq{q}")
                canq[q] = pp.tile([P, G * B], F32, tag=f"can{q}",
                                  name=f"canq{q}")

            def emit_wprep(wpool):
                # edge weights, computed once (in column halves to bound
                # SBUF); emitted lazily (before the first multiply) so no
                # early gather waits on it through Tile's cross-engine
                # clock alignment
                half = (FT + 1) // 2
                for h0 in (0, half):
                    h1 = min(h0 + half, FT)
                    hw = h1 - h0
                    th = wpool.tile([P, half], F32, tag="th")
                    sg = wpool.tile([P, half], F32, tag="sg")
                    cf = wpool.tile([P, half], F32, tag="cf")
                    dl = wpool.tile([P, half], F32, tag="dl")
                    mk = wpool.tile([P, half], I8, tag="mk")
                    nc.scalar.dma_start(out=th[:, :hw], in_=thD[:, h0:h1])
                    nc.scalar.dma_start(out=sg[:, :hw], in_=sgD[:, h0:h1])
                    nc.scalar.dma_start(out=cf[:, :hw], in_=cfD[:, h0:h1])
                    nc.scalar.dma_start(out=dl[:, :hw], in_=dlD[:, h0:h1])
                    nc.scalar.dma_start(out=mk[:, :hw], in_=mkD[:, h0:h1])
                    t = wpool.tile([P, half], F32, tag="t")
                    ws = w[:, h0:h1]
                    nc.scalar.activation(t[:, :hw], th[:, :hw], Tanh)
                    nc.vector.tensor_mul(ws, sg[:, :hw], cf[:, :hw])
                    nc.vector.copy_predicated(ws, mk[:, :hw], t[:, :hw])
                    nc.vector.tensor_mul(ws, ws, dl[:, :hw])

            DELAY = 4
            with (tc.tile_pool(name="wprep", bufs=1) as wpool,
                  tc.tile_pool(name="msgs", bufs=12) as mp):
                state = {"w": False, "ms": set()}

                def do_vector(q, ci, m):
                    if not state["w"]:
                        emit_wprep(wpool)
                        state["w"] = True
                    if q > 0 and int(plan["Gact"][q]) < G and q not in \
                            state["ms"]:
                        nc.vector.memset(aggq[q][:], 0.0)
                        state["ms"].add(q)
                    g0, g1, cols = chunks[q][ci]
                    c0, c1 = int(S[q, g0]), int(S[q, g1])
                    m3 = m[:, :cols * B].rearrange("p (s b) -> p s b", b=B)
                    w0 = int(Scol[q])
                    wb = (w[:, w0 + c0:w0 + c1].unsqueeze(-1)
                          .to_broadcast([P, cols, B]))
                    nc.vector.tensor_mul(m3, m3, wb)
                    aggt = agg0 if q == 0 else aggq[q]
                    for (ga, gb2, d) in _equal_d_runs(D[q], g0, g1):
                        src_ap = (m[:, (int(S[q, ga]) - c0) * B:
                                    (int(S[q, gb2]) - c0) * B]
                                  .rearrange("p (n d b) -> p n b d",
                                             d=d, b=B))
                        dst_ap = aggt[:, ga * B:gb2 * B].rearrange(
                            "p (n b) -> p n b", b=B)
                        nc.vector.tensor_reduce(
                            dst_ap, src_ap, axis=mybir.AxisListType.X,
                            op=mybir.AluOpType.add)
                    if q > 0:
                        # stage this chunk's groups to HBM for the re-gather
                        wg0, wg1 = g0, g1
                        if ci == len(chunks[q]) - 1 and int(
                                plan["Gact"][q]) < G:
                            wg1 = G          # include memset-zero tail groups
                        p3 = pagg[q - 1][:, :B].rearrange(
                            "(p g) b -> p g b", p=P)
                        nc.sync.dma_start(
                            out=p3[:, wg0:wg1, :],
                            in_=aggq[q][:, wg0 * B:wg1 * B].rearrange(
                                "p (g b) -> p g b", b=B))

                vqueue = []
                for kind, q, ci, qu in sched:
                    if kind == "g":
                        g0, g1, cols = chunks[q][ci]
                        slots = cols * P
                        m = mp.tile([P, CALL_COLS * B], F32, tag="m")
                        m3 = m[:, :cols * B].rearrange(
                            "p (s b) -> p s b", b=B)
                        base = q * SRC_CHUNK
                        in_ap = xT4[base:min(base + SRC_CHUNK, n_pad), :B]
                        gb = gcol0[(q, ci)]
                        if gb < spl:
                            idx_ap = gidxA[:, gb:gb + slots // 16]
                        else:
                            idx_ap = gidxB[:, gb - spl:gb - spl + slots // 16]
                        _raw_dma_gather(
                            nc.gpsimd, m3, in_ap, idx_ap,
                            slots, nreg[slots], B, ROWE, qu)
                        vqueue.append((q, ci, m))
                        if len(vqueue) > DELAY:
                            do_vector(*vqueue.pop(0))
                    else:
                        # canonical-order merge gather of the staged agg
                        g0, g1 = sblocks[q][ci]
                        nact = (g1 - g0) * P
                        c3 = canq[q][:, g0 * B:g1 * B].rearrange(
                            "p (g b) -> p g b", b=B)
                        sb = scol0[(q, ci)]
                        _raw_dma_gather(
                            nc.gpsimd, c3, pagg[q - 1][:, :B],
                            sidx_t[:, sb:sb + nact // 16],
                            nact, nreg[nact], B, ROWE, qu)
                for item in vqueue:
                    do_vector(*item)

            # ---- merge + ODE epilogue, chunked per merge-block range so
            # early ranges run under the final drains ----
            rate = pp.tile([P, G], F32, tag="rate")
            nc.scalar.activation(rate[:], rl[:], Exp)
            nc.vector.tensor_mul(rate[:], rate[:], br[:])
            dr = pp.tile([P, G * B], F32, tag="dr")

            for (g0, g1) in sblocks[1]:
                gw = g1 - g0
                sl = slice(g0 * B, g1 * B)
                for q in range(1, nq):
                    nc.vector.tensor_add(agg0[:, sl], agg0[:, sl],
                                         canq[q][:, sl])
                a3 = agg0[:, sl].rearrange("p (g b) -> p g b", b=B)
                bib = (bi[:, g0:g1].unsqueeze(-1)
                       .to_broadcast([P, gw, B]))
                cpb = (cp[:, g0:g1].unsqueeze(-1)
                       .to_broadcast([P, gw, B]))
                rateb = (rate[:, g0:g1].unsqueeze(-1)
                         .to_broadcast([P, gw, B]))
                d3 = dr[:, sl].rearrange("p (g b) -> p g b", b=B)
                nc.vector.tensor_add(d3, a3, bib)
                nc.scalar.activation(dr[:, sl], dr[:, sl], Tanh)
                nc.vector.tensor_mul(d3, d3, cpb)
                nc.vector.tensor_tensor(out=dr[:, sl], in0=dr[:, sl],
                                        in1=xo[:, sl],
                                        op=mybir.AluOpType.subtract)
                nc.vector.tensor_mul(d3, d3, rateb)
                nc.vector.tensor_scalar_mul(dr[:, sl], dr[:, sl], float(DT))
                nc.vector.tensor_add(dr[:, sl], dr[:, sl], xo[:, sl])
                nc.vector.tensor_scalar_max(dr[:, sl], dr[:, sl], 0.0)
                nc.vector.tensor_tensor(out=d3, in0=d3, in1=cpb,
                                        op=mybir.AluOpType.min)
                nc.sync.dma_start(out=outD[:, sl], in_=dr[:, sl])

    nc.compile()
    return nc


# ---------------------------------------------------------------------------
# Entry point
# ---------------------------------------------------------------------------

def kernel(x, theta_graph, node_bias, rate_log_scale, base_rate, capacity,
           sign_prior, conf_scale, delay_scale, src_index, dst_index,
           learn_mask):
    ins, plan = _prep(x, theta_graph, node_bias, rate_log_scale, base_rate,
                      capacity, sign_prior, conf_scale, delay_scale,
                      src_index, dst_index, learn_mask, NCORES)
    nc = build(plan)
    res = run_bass_kernel_spmd(nc, ins, core_ids=list(range(NCORES)))
    return _assemble(res.results, plan)


# revision 40
# speedup vs baseline: 1.0143x; 1.0143x over previous
"""Trainium2 Bass kernel for DifferentiableSupergraphDynamics.

Computation:
    edge_w = where(learn_mask, tanh(theta), sign*conf) * delay      [E]
    msgs   = x[:, src] * edge_w                                     [B, E]
    agg    = scatter_add(msgs -> dst)                               [B, N]
    rate   = base_rate * exp(rate_log_scale)                        [N]
    drive  = tanh(agg + bias)
    x_next = clip(x + DT * rate * (drive*cap - x), 0, cap)

Sharding: destination nodes are dealt round-robin (by total in-degree
rank) across the 8 cores; every edge lives on its destination's core, so
no cross-core collective is needed.

Per-core edge phase: edges are split into 4 "structures" by source-node
range (32768 rows each, so dma_gather's int16 indices can address the x
table). Each structure is a padded CSR over the core's nodes sorted by
that structure's in-degree: node groups of 128 partitions padded to the
group max degree D. Structures are processed as group-aligned chunks of
<= 8192 slots; each chunk is one SWDGE dma_gather call. Calls round-robin
the 4 SWDGE queues; with the descriptor ring doubled (32KB carveout) a
queue's next generation overlaps the previous call's drain, keeping all
8 Q7 cores generating descriptors continuously (~66us per 8192-desc call
per queue). Per-chunk Vector work (weight multiply + strided
tensor_reduce) and the per-structure merge scatter-adds (into canonical
node order via HBM) ride under the SWDGE generation critical path.
"""

import os

import numpy as np

# insurance against a wedged device left by a previous process
os.environ.setdefault("NEURON_RT_RESET_CORES", "1")

import concourse.bass as bass
import concourse.bacc as bacc
import concourse.mybir as mybir
import concourse.tile as tile
from concourse.bass_utils import run_bass_kernel_spmd

P = 128
NCORES = 8
DT = 0.1
SRC_CHUNK = 32768          # dma_gather int16 index reach
CALL_SLOTS = 8192          # max slots per gather call (<= ring capacity)
CALL_COLS = CALL_SLOTS // P
RING_BYTES = 32768         # SWDGE descriptor carveout (2 calls in flight)
ROWE = 64                  # x-table row stride: 256B (dma_gather req)

F32 = mybir.dt.float32
I16 = mybir.dt.int16
I8 = mybir.dt.int8


def _wrap_idx(flat):
    """SWDGE wrapped index layout for one call: index j at [j%16, j//16],
    replicated to 128 partitions (each queue's Q7 pair reads its own
    16-partition stripe)."""
    n = len(flat)
    assert n % 16 == 0
    cols = flat.reshape(n // 16, 16).T            # [16, n/16]
    return np.concatenate([cols] * 8, axis=0)     # [128, n/16]


# ---------------------------------------------------------------------------
# Host-side data preparation
# ---------------------------------------------------------------------------

def _chunk_groups(D_q, gact):
    """Split active groups [0, gact) into runs with sum(D) <= CALL_COLS."""
    chunks = []
    g = 0
    while g < gact:
        g2 = g
        tot = 0
        while g2 < gact and tot + D_q[g2] <= CALL_COLS:
            tot += D_q[g2]
            g2 += 1
        assert g2 > g
        chunks.append((g, g2, int(tot)))
        g = g2
    return chunks


def _prep(x, theta, bias, ratelog, baserate, cap, sign, conf, delay, src, dst,
          mask, n_cores):
    B, N = x.shape
    E = src.shape[0]

    src = np.asarray(src).astype(np.int64)
    dst = np.asarray(dst).astype(np.int64)
    theta = np.asarray(theta, dtype=np.float32)
    sign = np.asarray(sign, dtype=np.float32)
    conf = np.asarray(conf, dtype=np.float32)
    delay = np.asarray(delay, dtype=np.float32)
    mask8 = np.asarray(mask).astype(np.int8)

    deg = np.bincount(dst, minlength=N)
    order = np.argsort(-deg, kind="stable")
    npc = (N + n_cores - 1) // n_cores
    G = (npc + P - 1) // P
    nper = G * P                                   # nodes per core (padded)

    rank_of = np.empty(N, dtype=np.int64)
    rank_of[order] = np.arange(N)
    core_of = rank_of % n_cores                    # node -> core
    pos_of = rank_of // n_cores                    # node -> position in core

    n_pad = ((N + ROWE - 1) // ROWE) * ROWE
    nq = (n_pad + SRC_CHUNK - 1) // SRC_CHUNK     # structures
    q_of = src // SRC_CHUNK                        # edge -> structure

    # per (core, structure) in-degree
    edge_core = core_of[dst]
    edge_pos = pos_of[dst]
    degq = np.zeros((n_cores, nper, nq), dtype=np.int64)
    np.add.at(degq, (edge_core, edge_pos, q_of), 1)

    # shared-over-cores placement per structure: within each core sort
    # positions by degq desc; group windows of 128; D = max over cores.
    D = np.zeros((nq, G), dtype=np.int64)
    ordq = np.zeros((n_cores, nq, nper), dtype=np.int64)   # row j -> position
    invq = np.zeros((n_cores, nq, nper), dtype=np.int64)   # position -> row j
    for q in range(nq):
        for c in range(n_cores):
            o = np.argsort(-degq[c, :, q], kind="stable")
            ordq[c, q] = o
            invq[c, q, o] = np.arange(nper)
            dm = degq[c, o, q].reshape(G, P).max(axis=1)
            D[q] = np.maximum(D[q], dm)
    D[0] = np.maximum(D[0], 1)       # canonical layout covers all nodes
    S = np.zeros((nq, G + 1), dtype=np.int64)
    S[:, 1:] = np.cumsum(D, axis=1)
    F = S[:, -1]                                   # cols per structure
    Gact = np.array([int((D[q] > 0).sum()) for q in range(nq)])

    # --- edge slot assignment ---
    eord = np.lexsort((src, dst))
    ec = edge_core[eord]
    ep = edge_pos[eord]
    eq = q_of[eord]
    key_change = np.ones(E, dtype=bool)
    key_change[1:] = (dst[eord][1:] != dst[eord][:-1]) | (eq[1:] != eq[:-1])
    run_id = np.cumsum(key_change) - 1
    run_starts = np.flatnonzero(key_change)
    occ = np.arange(E) - run_starts[run_id]

    row = invq[ec, eq, ep]                         # row index in structure
    g = row // P
    pp = row % P
    col = S[eq, g] + occ
    slot_i = pp + P * col                          # slot within (core, struct)

    FT = int(F.sum())
    Scol = np.zeros(nq + 1, dtype=np.int64)
    Scol[1:] = np.cumsum(F)

    # edge params laid out [P, FT] per core (slot (q,p,col) -> [p,Scol[q]+col])
    par_shape = (n_cores, P, FT)
    thetaA = np.zeros(par_shape, np.float32)
    signA = np.zeros(par_shape, np.float32)
    confA = np.zeros(par_shape, np.float32)
    delayA = np.zeros(par_shape, np.float32)
    maskA = np.zeros(par_shape, np.int8)
    pidx = (ec, pp, Scol[eq] + col)
    thetaA[pidx] = theta[eord]
    signA[pidx] = sign[eord]
    confA[pidx] = conf[eord]
    delayA[pidx] = delay[eord]
    maskA[pidx] = mask8[eord]

    # --- chunk plans (shared across cores: D is shared) ---
    chunks = [_chunk_groups(D[q], int(Gact[q])) for q in range(nq)]



    # --- merge-gather blocks: canonical-order re-gather of each partial
    # aggregate, split into ~25-col blocks (slot counts multiple of 128) ---
    def _merge_blocks(gtot):
        nb = max(1, (gtot + 24) // 25)
        base, rem = divmod(gtot, nb)
        out, g = [], 0
        for i in range(nb):
            w = base + (1 if i < rem else 0)
            out.append((g, g + w))
            g += w
        return out

    sblocks = {q: _merge_blocks(G) for q in range(1, nq)}

    # emission schedule: gathers for structures in qorder, with each
    # structure's merge-scatter chunks emitted after the NEXT structure's
    # gathers (so their reduce deps are long satisfied at dispatch).
    # qorder = [1, 2, 3, 0]; scatter(1) after G2, scatter(3) after G3... etc.
    # gather emission order: structures 1, 2, 3, 0
    gorder = ([("g", 1, i) for i in range(len(chunks[1]))] +
              [("g", 2, i) for i in range(len(chunks[2]))] +
              [("g", 3, i) for i in range(len(chunks[3]))] +
              [("g", 0, i) for i in range(len(chunks[0]))])
    gpos = {(q, ci): j for j, (_, q, ci) in enumerate(gorder)}

    # merge-gather blocks depend on the whole structure's reduces plus its
    # agg HBM writeback; emit them LAG gathers after the structure's last
    # chunk so those (which trail the drain by ~1 round) never stall SEQ
    # dispatch.
    LAG = 8
    pend = []
    for q in range(1, nq):
        last = gpos[(q, len(chunks[q]) - 1)]
        for bi in range(len(sblocks[q])):
            pend.append((last + LAG + bi, ("s", q, bi)))
    pend.sort(key=lambda e: e[0])

    sched = []                                     # (kind, q, idx, queue)
    pi = 0
    load = [0, 0, 0, 0]

    def _emit(ev):
        kind, q, ci = ev
        if kind == "g":
            n = chunks[q][ci][2] * P
        else:
            g0, g1 = sblocks[q][ci]
            n = (g1 - g0) * P
        qu = min(range(4), key=lambda r: (load[r], r))
        load[qu] += n
        sched.append((kind, q, ci, qu))

    for j, gev in enumerate(gorder):
        while pi < len(pend) and pend[pi][0] <= j:
            _emit(pend[pi][1])
            pi += 1
        _emit(gev)
    for _, ev in pend[pi:]:
        _emit(ev)

    # --- per-core gather index blobs, laid out in EMISSION order so the
    # first rounds' indices can be loaded (and gathered from) first ---
    srcrel = (src[eord] - eq * SRC_CHUNK).astype(np.int16)
    amaps = {}
    for q in range(nq):
        tots = int(F[q]) * P
        for c in range(n_cores):
            a = np.zeros(tots, np.int16)
            selq = (ec == c) & (eq == q)
            a[slot_i[selq]] = srcrel[selq]
            amaps[(q, c)] = a
    gcol0 = {}                                     # (q, ci) -> gidx col offset
    gidx_parts = [[] for _ in range(n_cores)]
    colp = 0
    gidx_split = 0
    for j, (_, q, ci) in enumerate(gorder):
        g0, g1, cols = chunks[q][ci]
        c0, c1 = int(S[q, g0]), int(S[q, g1])
        gcol0[(q, ci)] = colp
        for c in range(n_cores):
            gidx_parts[c].append(_wrap_idx(amaps[(q, c)][c0 * P:c1 * P]))
        colp += (c1 - c0) * P // 16
        if j == 3:
            gidx_split = colp                      # first round loads alone
    gidx = [np.concatenate(p, axis=1) for p in gidx_parts]
    gidx_cols = gidx[0].shape[1]

    # --- per-core merge-gather index blobs: for canonical output slot
    # (p, g) the structure-q table row of its node; table row layout is
    # (jq % P) * G + jq // P where jq is the node's structure-q row ---
    scol0 = {}
    sidx_parts = [[] for _ in range(n_cores)]
    colp = 0
    qrow_of = np.zeros((n_cores, nq, nper), np.int16)   # canonical row->q row
    for q in range(1, nq):
        for c in range(n_cores):
            node_pos = ordq[c, 0]                  # canonical row -> position
            jq = invq[c, q, node_pos]              # -> structure-q row
            qrow_of[c, q] = ((jq % P) * G + (jq // P)).astype(np.int16)
    for q in range(1, nq):
        for ci, (g0, g1) in enumerate(sblocks[q]):
            scol0[(q, ci)] = colp
            nact = (g1 - g0) * P
            for c in range(n_cores):
                # output slot s = p + 128*(g-g0) -> canonical row g*P + p,
                # which is arange(g0*P, g1*P) in slot order
                crow = np.arange(g0 * P, g1 * P)
                sidx_parts[c].append(_wrap_idx(qrow_of[c, q, crow]))
            colp += nact // 16
    sidx = [np.concatenate(p, axis=1) for p in sidx_parts]
    sidx_cols = sidx[0].shape[1]

    # node params in canonical placement [P, G]
    def node_arr(vals, fill):
        a = np.full((n_cores, P, G), fill, dtype=np.float32)
        for c in range(n_cores):
            node_pos = ordq[c, 0]
            rank = node_pos * n_cores + c
            ok = rank < N
            nd = order[np.minimum(rank, N - 1)]
            v = np.where(ok, vals[nd], fill).astype(np.float32)
            a[c].reshape(-1)[(np.arange(nper) % P) * G +
                             (np.arange(nper) // P)] = np.where(ok, v, fill)
        return a

    biasA = node_arr(np.asarray(bias), 0.0)
    ratelogA = node_arr(np.asarray(ratelog), 0.0)
    baserateA = node_arr(np.asarray(baserate), 0.0)
    capA = node_arr(np.asarray(cap), 1.0)

    xT4 = np.zeros((n_pad, ROWE), np.float32)
    xT4[:N, :B] = np.asarray(x, dtype=np.float32).T

    xTf = xT4[:, :B]
    xownA = np.zeros((n_cores, P, G, B), np.float32)
    node_ids = np.zeros((n_cores, P, G), np.int64)
    for c in range(n_cores):
        node_pos = ordq[c, 0]
        rank = node_pos * n_cores + c
        ok = rank < N
        nd = np.where(ok, order[np.minimum(rank, N - 1)], -1)
        jj = np.arange(nper)
        pcol = (jj % P, jj // P)
        node_ids[c][pcol] = nd
        xownA[c][pcol[0], pcol[1], :] = np.where(
            ok[:, None], xTf[np.maximum(nd, 0), :], 0.0)

    ins = []
    for c in range(n_cores):
        ins.append({
            "xT4": xT4,
            "gidx": gidx[c],
            "sidx": sidx[c],
            "theta": thetaA[c],
            "sgn": signA[c],
            "conf": confA[c],
            "delay": delayA[c],
            "maskf": maskA[c],
            "bias": biasA[c],
            "ratelog": ratelogA[c],
            "baserate": baserateA[c],
            "cap": capA[c],
            "xown": xownA[c].reshape(P, G * B),
        })
    plan = dict(B=B, N=N, G=G, nq=nq, D=D, S=S, F=F, Scol=Scol, Gact=Gact,
                n_pad=n_pad, gidx_cols=gidx_cols, sidx_cols=sidx_cols,
                chunks=chunks, sblocks=sblocks, sched=sched, gcol0=gcol0,
                scol0=scol0, gidx_split=gidx_split, node_ids=node_ids)
    return ins, plan


def _assemble(results, plan):
    B, N, G = plan["B"], plan["N"], plan["G"]
    out = np.empty((B, N), dtype=np.float32)
    for ci, res in enumerate(results):
        o = res["out"].reshape(P, G, B)
        nid = plan["node_ids"][ci]
        ok = nid >= 0
        out[:, nid[ok]] = o[ok].T
    return out


# ---------------------------------------------------------------------------
# Device kernel
# ---------------------------------------------------------------------------

def _raw_dma_gather(g, out_ap, in_ap, idxs_ap, num_idxs, num_idxs_reg,
                    elem_size, elem_step, queue_num):
    stride_bytes = elem_step * mybir.dt.size(in_ap.dtype)
    return g.add_instruction(
        mybir.InstDMAGatherAnt(
            name=g.bass.get_next_instruction_name(),
            ins=[*g.lower_ap_dma(in_ap, for_custom_bir_dma=True),
                 g.lower_ap(idxs_ap), g.lower_val_access(num_idxs_reg)],
            outs=[g.lower_ap(out_ap)],
            transpose=False, num_idxs=num_idxs, elem_size=elem_size,
            stride_bytes_256=stride_bytes // 256, gen_mode=0,
            single_packet=False, queue_num=queue_num,
            sbuf_tokens_per_rank=0, sbuf_free_dim_per_rank=0,
            sbuf_free_dim_pad_per_rank=0, sbuf_byte_offset=0))


def _equal_d_runs(D, g0, g1):
    runs = []
    a = g0
    while a < g1:
        b = a + 1
        while b < g1 and D[b] == D[a]:
            b += 1
        runs.append((a, b, int(D[a])))
        a = b
    return runs


def build(plan):
    B = plan["B"]
    G = plan["G"]
    nq = plan["nq"]
    D, S, F, Scol = plan["D"], plan["S"], plan["F"], plan["Scol"]
    n_pad = plan["n_pad"]
    chunks, sched = plan["chunks"], plan["sched"]
    sblocks = plan["sblocks"]
    gcol0, scol0 = plan["gcol0"], plan["scol0"]
    FT = int(Scol[-1])

    # distinct SWDGE call sizes -> one shared register each (written once;
    # per-call to_reg MOVEs rewrite a single GPR and Tile serializes every
    # call behind the previous one's completion sem to protect it)
    sizes = set()
    for kind, q, ci, _qu in sched:
        if kind == "g":
            sizes.add(chunks[q][ci][2] * P)
        else:
            g0, g1 = sblocks[q][ci]
            sizes.add((g1 - g0) * P)

    nc = bacc.Bacc("TRN2", target_bir_lowering=False, debug=False,
                   enable_asserts=False, num_swdge_queues=4,
                   dynamic_dma_scratch_size=RING_BYTES)

    xT4 = nc.dram_tensor("xT4", [n_pad, ROWE], F32, kind="ExternalInput")
    giD = nc.dram_tensor("gidx", [128, plan["gidx_cols"]], I16,
                         kind="ExternalInput")
    siD = nc.dram_tensor("sidx", [128, plan["sidx_cols"]], I16,
                         kind="ExternalInput")
    thD = nc.dram_tensor("theta", [P, FT], F32, kind="ExternalInput")
    sgD = nc.dram_tensor("sgn", [P, FT], F32, kind="ExternalInput")
    cfD = nc.dram_tensor("conf", [P, FT], F32, kind="ExternalInput")
    dlD = nc.dram_tensor("delay", [P, FT], F32, kind="ExternalInput")
    mkD = nc.dram_tensor("maskf", [P, FT], I8, kind="ExternalInput")
    biD = nc.dram_tensor("bias", [P, G], F32, kind="ExternalInput")
    rlD = nc.dram_tensor("ratelog", [P, G], F32, kind="ExternalInput")
    brD = nc.dram_tensor("baserate", [P, G], F32, kind="ExternalInput")
    cpD = nc.dram_tensor("cap", [P, G], F32, kind="ExternalInput")
    xoD = nc.dram_tensor("xown", [P, G * B], F32, kind="ExternalInput")
    outD = nc.dram_tensor("out", [P, G * B], F32, kind="ExternalOutput")
    # partial-agg merge staging tables (HBM roundtrip for the canonical
    # re-gather)
    pagg = [nc.dram_tensor(f"pagg{q}", [G * P, ROWE], F32,
                           kind="ExternalOutput") for q in range(1, nq)]

    Tanh = mybir.ActivationFunctionType.Tanh
    Exp = mybir.ActivationFunctionType.Exp

    with tile.TileContext(nc) as tc:
        with tc.tile_pool(name="persist", bufs=1) as pp:
            nreg = {v: nc.gpsimd.to_reg(v) for v in sorted(sizes)}

            # first round's gather indices load alone so gathers start early
            spl = int(plan["gidx_split"])
            gidxA = pp.tile([128, spl], I16, tag="gidxA")
            nc.sync.dma_start(out=gidxA[:], in_=giD[:, :spl])
            gidxB = pp.tile([128, plan["gidx_cols"] - spl], I16, tag="gidxB")
            nc.sync.dma_start(out=gidxB[:], in_=giD[:, spl:])
            sidx_t = pp.tile([128, plan["sidx_cols"]], I16, tag="sidx")
            nc.sync.dma_start(out=sidx_t[:], in_=siD[:, :])

            bi = pp.tile([P, G], F32, tag="bi")
            rl = pp.tile([P, G], F32, tag="rl")
            br = pp.tile([P, G], F32, tag="br")
            cp = pp.tile([P, G], F32, tag="cp")
            xo = pp.tile([P, G * B], F32, tag="xo")
            nc.sync.dma_start(out=bi[:], in_=biD[:, :])
            nc.sync.dma_start(out=rl[:], in_=rlD[:, :])
            nc.sync.dma_start(out=br[:], in_=brD[:, :])
            nc.sync.dma_start(out=cp[:], in_=cpD[:, :])
            nc.sync.dma_start(out=xo[:], in_=xoD[:, :])

            w = pp.tile([P, FT], F32, tag="w")
            agg0 = pp.tile([P, G * B], F32, tag="agg0")
            aggq = {}
            canq = {}
            for q in range(1, nq):
                aggq[q] = pp.tile([P, G * B], F32, tag=f"agg{q}",
                                  name=f"aggq{q}")
                canq[q] = pp.tile([P, G * B], F32, tag=f"can{q}",
                                  name=f"canq{q}")

            def emit_wprep(wpool):
                # edge weights, computed once (in column halves to bound
                # SBUF); emitted lazily (before the first multiply) so no
                # early gather waits on it through Tile's cross-engine
                # clock alignment
                half = (FT + 1) // 2
                for h0 in (0, half):
                    h1 = min(h0 + half, FT)
                    hw = h1 - h0
                    th = wpool.tile([P, half], F32, tag="th")
                    sg = wpool.tile([P, half], F32, tag="sg")
                    cf = wpool.tile([P, half], F32, tag="cf")
                    dl = wpool.tile([P, half], F32, tag="dl")
                    mk = wpool.tile([P, half], I8, tag="mk")
                    nc.scalar.dma_start(out=th[:, :hw], in_=thD[:, h0:h1])
                    nc.scalar.dma_start(out=sg[:, :hw], in_=sgD[:, h0:h1])
                    nc.scalar.dma_start(out=cf[:, :hw], in_=cfD[:, h0:h1])
                    nc.scalar.dma_start(out=dl[:, :hw], in_=dlD[:, h0:h1])
                    nc.scalar.dma_start(out=mk[:, :hw], in_=mkD[:, h0:h1])
                    t = wpool.tile([P, half], F32, tag="t")
                    ws = w[:, h0:h1]
                    nc.scalar.activation(t[:, :hw], th[:, :hw], Tanh)
                    nc.vector.tensor_mul(ws, sg[:, :hw], cf[:, :hw])
                    nc.vector.copy_predicated(ws, mk[:, :hw], t[:, :hw])
                    nc.vector.tensor_mul(ws, ws, dl[:, :hw])

            DELAY = 4
            with (tc.tile_pool(name="wprep", bufs=1) as wpool,
                  tc.tile_pool(name="msgs", bufs=12) as mp):
                state = {"w": False, "ms": set()}

                def do_vector(q, ci, m):
                    if not state["w"]:
                        emit_wprep(wpool)
                        state["w"] = True
                    if q > 0 and int(plan["Gact"][q]) < G and q not in \
                            state["ms"]:
                        nc.vector.memset(aggq[q][:], 0.0)
                        state["ms"].add(q)
                    g0, g1, cols = chunks[q][ci]
                    c0, c1 = int(S[q, g0]), int(S[q, g1])
                    m3 = m[:, :cols * B].rearrange("p (s b) -> p s b", b=B)
                    w0 = int(Scol[q])
                    wb = (w[:, w0 + c0:w0 + c1].unsqueeze(-1)
                          .to_broadcast([P, cols, B]))
                    nc.vector.tensor_mul(m3, m3, wb)
                    aggt = agg0 if q == 0 else aggq[q]
                    for (ga, gb2, d) in _equal_d_runs(D[q], g0, g1):
                        src_ap = (m[:, (int(S[q, ga]) - c0) * B:
                                    (int(S[q, gb2]) - c0) * B]
                                  .rearrange("p (n d b) -> p n b d",
                                             d=d, b=B))
                        dst_ap = aggt[:, ga * B:gb2 * B].rearrange(
                            "p (n b) -> p n b", b=B)
                        nc.vector.tensor_reduce(
                            dst_ap, src_ap, axis=mybir.AxisListType.X,
                            op=mybir.AluOpType.add)
                    if q > 0:
                        # stage this chunk's groups to HBM for the re-gather
                        wg0, wg1 = g0, g1
                        if ci == len(chunks[q]) - 1 and int(
                                plan["Gact"][q]) < G:
                            wg1 = G          # include memset-zero tail groups
                        p3 = pagg[q - 1][:, :B].rearrange(
                            "(p g) b -> p g b", p=P)
                        nc.sync.dma_start(
                            out=p3[:, wg0:wg1, :],
                            in_=aggq[q][:, wg0 * B:wg1 * B].rearrange(
                                "p (g b) -> p g b", b=B))

                vqueue = []
                for kind, q, ci, qu in sched:
                    if kind == "g":
                        g0, g1, cols = chunks[q][ci]
                        slots = cols * P
                        m = mp.tile([P, CALL_COLS * B], F32, tag="m")
                        m3 = m[:, :cols * B].rearrange(
                            "p (s b) -> p s b", b=B)
                        base = q * SRC_CHUNK
                        in_ap = xT4[base:min(base + SRC_CHUNK, n_pad), :B]
                        gb = gcol0[(q, ci)]
                        if gb < spl:
                            idx_ap = gidxA[:, gb:gb + slots // 16]
                        else:
                            idx_ap = gidxB[:, gb - spl:gb - spl + slots // 16]
                        _raw_dma_gather(
                            nc.gpsimd, m3, in_ap, idx_ap,
                            slots, nreg[slots], B, ROWE, qu)
                        vqueue.append((q, ci, m))
                        if len(vqueue) > DELAY:
                            do_vector(*vqueue.pop(0))
                    else:
                        # canonical-order merge gather of the staged agg
                        g0, g1 = sblocks[q][ci]
                        nact = (g1 - g0) * P
                        c3 = canq[q][:, g0 * B:g1 * B].rearrange(
                            "p (g b) -> p g b", b=B)
                        sb = scol0[(q, ci)]
                        _raw_dma_gather(
                            nc.gpsimd, c3, pagg[q - 1][:, :B],
                            sidx_t[:, sb:sb + nact // 16],
                            nact, nreg[nact], B, ROWE, qu)
                for item in vqueue:
                    do_vector(*item)

            # ---- merge + ODE epilogue, chunked per merge-block range so
            # early ranges run under the final drains ----
            rate = pp.tile([P, G], F32, tag="rate")
            nc.scalar.activation(rate[:], rl[:], Exp)
            nc.vector.tensor_mul(rate[:], rate[:], br[:])
            dr = pp.tile([P, G * B], F32, tag="dr")

            for (g0, g1) in sblocks[1]:
                gw = g1 - g0
                sl = slice(g0 * B, g1 * B)
                for q in range(1, nq):
                    nc.vector.tensor_add(agg0[:, sl], agg0[:, sl],
                                         canq[q][:, sl])
                a3 = agg0[:, sl].rearrange("p (g b) -> p g b", b=B)
                bib = (bi[:, g0:g1].unsqueeze(-1)
                       .to_broadcast([P, gw, B]))
                cpb = (cp[:, g0:g1].unsqueeze(-1)
                       .to_broadcast([P, gw, B]))
                rateb = (rate[:, g0:g1].unsqueeze(-1)
                         .to_broadcast([P, gw, B]))
                d3 = dr[:, sl].rearrange("p (g b) -> p g b", b=B)
                nc.vector.tensor_add(d3, a3, bib)
                nc.scalar.activation(dr[:, sl], dr[:, sl], Tanh)
                nc.vector.tensor_mul(d3, d3, cpb)
                nc.vector.tensor_tensor(out=dr[:, sl], in0=dr[:, sl],
                                        in1=xo[:, sl],
                                        op=mybir.AluOpType.subtract)
                nc.vector.tensor_mul(d3, d3, rateb)
                nc.vector.tensor_scalar_mul(dr[:, sl], dr[:, sl], float(DT))
                nc.vector.tensor_add(dr[:, sl], dr[:, sl], xo[:, sl])
                nc.vector.tensor_scalar_max(dr[:, sl], dr[:, sl], 0.0)
                nc.vector.tensor_tensor(out=d3, in0=d3, in1=cpb,
                                        op=mybir.AluOpType.min)
                nc.sync.dma_start(out=outD[:, sl], in_=dr[:, sl])

    nc.compile()
    return nc


# ---------------------------------------------------------------------------
# Entry point
# ---------------------------------------------------------------------------

def kernel(x, theta_graph, node_bias, rate_log_scale, base_rate, capacity,
           sign_prior, conf_scale, delay_scale, src_index, dst_index,
           learn_mask):
    ins, plan = _prep(x, theta_graph, node_bias, rate_log_scale, base_rate,
                      capacity, sign_prior, conf_scale, delay_scale,
                      src_index, dst_index, learn_mask, NCORES)
    nc = build(plan)
    res = run_bass_kernel_spmd(nc, ins, core_ids=list(range(NCORES)))
    return _assemble(res.results, plan)
